# revision 5
# baseline (speedup 1.0000x reference)
"""MaxSim InfoNCE loss on 8 Trainium2 NeuronCores.

Strategy
--------
sim[b1,b2,t,i] = <text[b1,t], image[b2,i]>; logits = mean_t max_i sim / T;
loss = 0.5*(CE_diag(logits) + CE_diag(logits^T)).

Shard the image batch (b2) 8-way: each core holds the FULL text set and a
16-image shard, all resident in SBUF as bf16 in [d, row] (transposed) layout
prepared on the host.  Per core:
  * 64 text m-tiles x 8 col-tiles (392 cols = 2 images) x 4 K-chunks of
    bf16 matmuls accumulate sim blocks in PSUM,
  * DVE reduce_max per 196-col image segment -> maxvals[128, 1024] f32,
  * 64 small fp32 matmuls against a [128,2] block-ones matrix fold the
    mean over t (64 rows) and the 1/T scale -> logits^T block [16, 128],
  * AllGather the [16,128] blocks -> full logits^T [128,128] on every core,
  * on-chip CE both directions (PE transpose, ACT exp/ln with fused row
    sums, diag via identity mask) -> scalar loss.
"""

import numpy as np
import ml_dtypes

B = 128          # batch (both text and image)
TT = 64          # text tokens
II = 196         # image tokens
D = 512          # embed dim
NCORES = 8
IPC = B // NCORES          # images per core = 16
COLS = IPC * II            # 3136 sim columns per core
TEMP = 0.07
KCH = 4                    # K chunks of 128
MT = (B * TT) // 128       # 64 text m-tiles
CT = 8                     # col tiles per core (392 cols each)
CW = 2 * II                # 392

_CACHE = {}


def _build():
    import concourse.bacc as bacc
    import concourse.mybir as mybir
    from concourse import tile

    f32 = mybir.dt.float32
    bf16 = mybir.dt.bfloat16
    X = mybir.AxisListType.X
    Exp = mybir.ActivationFunctionType.Exp
    Ln = mybir.ActivationFunctionType.Ln

    nc = bacc.Bacc(
        "TRN2", target_bir_lowering=False, debug=False, num_devices=NCORES
    )

    txt_dram = nc.dram_tensor("text_t", [KCH, 128, B * TT], bf16, kind="ExternalInput")
    img_dram = nc.dram_tensor("img_t", [KCH, 128, COLS], bf16, kind="ExternalInput")
    out_dram = nc.dram_tensor("loss", [1, 1], f32, kind="ExternalOutput")

    ident_np = np.eye(128, dtype=np.float32)
    ones2_np = np.zeros((128, 2), dtype=np.float32)
    ones2_np[0:64, 0] = 1.0 / (TT * TEMP)
    ones2_np[64:128, 1] = 1.0 / (TT * TEMP)
    half_np = np.full((128, 1), 0.5 / B, dtype=np.float32)
    ident_d = nc.inline_tensor(ident_np, "ident_c")
    ones2_d = nc.inline_tensor(ones2_np, "ones2_c")
    half_d = nc.inline_tensor(half_np, "half_c")

    with tile.TileContext(nc) as tc:
        with (
            tc.tile_pool(name="const", bufs=1) as constp,
            tc.tile_pool(name="data", bufs=1) as datap,
            tc.tile_pool(name="mx", bufs=1) as mxp,
            tc.tile_pool(name="work", bufs=1) as workp,
            tc.tile_pool(name="pmain", bufs=3, space="PSUM") as pmain,
            tc.tile_pool(name="pmisc", bufs=1, space="PSUM") as pmisc,
            tc.tile_pool(name="dram", bufs=1, space="DRAM") as dramp,
        ):
            ident = constp.tile([128, 128], f32, tag="ident", name="ident")
            nc.sync.dma_start(ident[:], ident_d[:])
            ones2 = constp.tile([128, 2], f32, tag="ones2", name="ones2")
            nc.sync.dma_start(ones2[:], ones2_d[:])
            half1 = constp.tile([128, 1], f32, tag="half1", name="half1")
            nc.sync.dma_start(half1[:], half_d[:])

            # resident tiles, DMA'd in first-use order: text g0, all image,
            # then remaining text (PE's first m-tile needs txt g0 + all img h)
            txtt = {}
            imgt = {}
            for k in range(KCH):
                t = datap.tile([128, 1024], bf16, tag=f"txt{k}g0", name=f"txt{k}g0")
                nc.sync.dma_start(t[:], txt_dram[k, :, 0:1024])
                txtt[k, 0] = t
            for h in range(CT):
                for k in range(KCH):
                    t = datap.tile(
                        [128, CW], bf16, tag=f"img{k}h{h}", name=f"img{k}h{h}"
                    )
                    nc.sync.dma_start(t[:], img_dram[k, :, CW * h : CW * (h + 1)])
                    imgt[k, h] = t
            for g in range(1, 8):
                for k in range(KCH):
                    t = datap.tile(
                        [128, 1024], bf16, tag=f"txt{k}g{g}", name=f"txt{k}g{g}"
                    )
                    nc.sync.dma_start(
                        t[:], txt_dram[k, :, 1024 * g : 1024 * (g + 1)]
                    )
                    txtt[k, g] = t

            maxv = mxp.tile([128, MT * IPC], f32, tag="maxv", name="maxv")
            # logits^T accumulator [16, 128]; written by interleaved mean-mms
            lgps = pmisc.tile([IPC, 128], f32, tag="misc", name="lgps")

            def mean_mm(m):
                # fold mean over t (and 1/T): [16,2] block of logits^T
                nc.tensor.matmul(
                    lgps[:, 2 * m : 2 * m + 2],
                    maxv[:, IPC * m : IPC * (m + 1)],
                    ones2[:],
                    start=True,
                    stop=True,
                )

            for m in range(MT):
                g, mo = divmod(m, 8)
                for cp in range(CT // 2):
                    # 2 full PSUM banks: 392-col image pair per bank (bank
                    # boundary at 512 f32 -- regions must not cross it)
                    ps = pmain.tile([128, 1024], f32, tag="ps", name="ps")
                    for r in range(2):
                        c = 2 * cp + r
                        for k in range(KCH):
                            nc.tensor.matmul(
                                ps[:, 512 * r : 512 * r + CW],
                                txtt[k, g][:, 128 * mo : 128 * (mo + 1)],
                                imgt[k, c][:],
                                start=(k == 0),
                                stop=(k == KCH - 1),
                            )
                    j = IPC * m + 4 * cp
                    nc.vector.reduce_max(
                        maxv[:, j : j + 4],
                        ps.rearrange("p (b r) -> p b r", b=2)[:, :, 0:CW]
                        .rearrange("p b (i x) -> p b i x", i=2),
                        axis=X,
                    )
                if m > 0:
                    mean_mm(m - 1)  # pipelined: maxv row-block m-1 is complete
            mean_mm(MT - 1)
            lgT_local = workp.tile([IPC, 128], f32, tag="lgT_local", name="lgT_local")
            nc.vector.tensor_copy(lgT_local[:], lgps[:])

            cc_in = dramp.tile([IPC, 128], f32, tag="cc_in", name="cc_in")
            cc_out = dramp.tile(
                [B, 128], f32, tag="cc_out", name="cc_out", addr_space="Shared"
            )
            nc.sync.dma_start(cc_in[:], lgT_local[:])
            nc.gpsimd.collective_compute(
                "AllGather",
                mybir.AluOpType.bypass,
                replica_groups=[list(range(NCORES))],
                ins=[cc_in.opt()],
                outs=[cc_out.opt()],
            )

            # full logits^T on every core -> scalar loss
            lgT = workp.tile([128, 128], f32, tag="lgT", name="lgT")
            nc.sync.dma_start(lgT[:], cc_out[:])
            ps_t = pmisc.tile([128, 128], f32, tag="misc", name="ps_t")
            nc.tensor.transpose(ps_t[:], lgT[:], ident[:])
            lg = workp.tile([128, 128], f32, tag="lg", name="lg")
            nc.vector.tensor_copy(lg[:], ps_t[:])

            def row_lse(src, nm):
                mx = workp.tile([128, 1], f32, tag=f"mx_{nm}", name=f"mx_{nm}")
                nc.vector.reduce_max(mx[:], src[:], axis=X)
                nmx = workp.tile([128, 1], f32, tag=f"nmx_{nm}", name=f"nmx_{nm}")
                nc.vector.tensor_scalar_mul(nmx[:], mx[:], -1.0)
                et = workp.tile([128, 128], f32, tag=f"et_{nm}", name=f"et_{nm}")
                sm = workp.tile([128, 1], f32, tag=f"sm_{nm}", name=f"sm_{nm}")
                nc.scalar.activation(
                    et[:], src[:], Exp, bias=nmx[:], scale=1.0, accum_out=sm[:]
                )
                ls = workp.tile([128, 1], f32, tag=f"ls_{nm}", name=f"ls_{nm}")
                nc.scalar.activation(ls[:], sm[:], Ln)
                lse = workp.tile([128, 1], f32, tag=f"lse_{nm}", name=f"lse_{nm}")
                nc.vector.tensor_sub(lse[:], ls[:], nmx[:])
                return lse

            lse_t2i = row_lse(lgT, "a")   # rows of logits^T: lse over b1
            lse_i2t = row_lse(lg, "b")    # rows of logits:   lse over b2

            dgt = workp.tile([128, 128], f32, tag="dgt", name="dgt")
            nc.vector.tensor_mul(dgt[:], lg[:], ident[:])
            dg = workp.tile([128, 1], f32, tag="dg", name="dg")
            nc.vector.reduce_sum(dg[:], dgt[:], axis=X)

            t_a = workp.tile([128, 1], f32, tag="t_a", name="t_a")
            nc.vector.tensor_add(t_a[:], lse_t2i[:], lse_i2t[:])
            t_b = workp.tile([128, 1], f32, tag="t_b", name="t_b")
            nc.vector.tensor_scalar_mul(t_b[:], dg[:], -2.0)
            rowterm = workp.tile([128, 1], f32, tag="rowterm", name="rowterm")
            nc.vector.tensor_add(rowterm[:], t_a[:], t_b[:])

            ps_l = pmisc.tile([1, 1], f32, tag="misc", name="ps_l")
            nc.tensor.matmul(ps_l[:], rowterm[:], half1[:], start=True, stop=True)
            loss_sb = workp.tile([1, 1], f32, tag="loss_sb", name="loss_sb")
            nc.vector.tensor_copy(loss_sb[:], ps_l[:])
            nc.sync.dma_start(out_dram[:], loss_sb[:])

    nc.compile()
    return nc


def _in_maps(image_tokens, text_tokens):
    txt = np.asarray(text_tokens, dtype=np.float32).reshape(B * TT, D)
    txtT = np.ascontiguousarray(txt.T).astype(ml_dtypes.bfloat16)
    text_t = txtT.reshape(KCH, 128, B * TT)
    img = np.asarray(image_tokens, dtype=np.float32)
    maps = []
    for c in range(NCORES):
        sh = img[IPC * c : IPC * (c + 1)].reshape(COLS, D)
        shT = np.ascontiguousarray(sh.T).astype(ml_dtypes.bfloat16)
        maps.append({"text_t": text_t, "img_t": shT.reshape(KCH, 128, COLS)})
    return maps


def run(image_tokens, text_tokens, trace=False):
    from concourse.bass_utils import run_bass_kernel_spmd

    if "nc" not in _CACHE:
        _CACHE["nc"] = _build()
    nc = _CACHE["nc"]
    res = run_bass_kernel_spmd(
        nc,
        _in_maps(image_tokens, text_tokens),
        core_ids=list(range(NCORES)),
        trace=trace,
    )
    return res


def kernel(image_tokens, text_tokens):
    res = run(image_tokens, text_tokens, trace=False)
    out = np.asarray(res.results[0]["loss"], dtype=np.float32).reshape(())
    return out


# revision 6
# speedup vs baseline: 1.3602x; 1.3602x over previous
"""MaxSim InfoNCE loss on 8 Trainium2 NeuronCores.

Strategy
--------
sim[b1,b2,t,i] = <text[b1,t], image[b2,i]>; logits = mean_t max_i sim / T;
loss = 0.5*(CE_diag(logits) + CE_diag(logits^T)).

Shard the image batch (b2) 8-way: each core holds the FULL text set and a
16-image shard, all resident in SBUF in [d, row] (transposed) layout
prepared on the host.  Per core:
  * 64 text m-tiles x 8 col-tiles (392 cols = 2 images), contraction D=512
    done as fp8-e4m3 DoubleRow matmuls (K=256/pass, 2 passes) into PSUM,
  * DVE reduce_max per 196-col image segment (4 images per 2-bank PSUM
    tile, single 4D-AP reduce) -> maxvals[128, 1024] f32,
  * 64 small fp32 matmuls against a [128,2] block-ones matrix fold the
    mean over t (64 rows) and the 1/T scale -> logits^T block [16, 128],
  * AllGather the [16,128] blocks -> full logits^T [128,128] on every core,
  * on-chip CE both directions (PE transpose, ACT exp/ln with fused row
    sums, diag via identity mask) -> scalar loss.

MODE selects the matmul dtype: "fp8dr" (DoubleRow, ~5e-4 rel err) or
"bf16" (~7e-5 rel err, ~2x slower PE).
"""

import numpy as np
import ml_dtypes

B = 128          # batch (both text and image)
TT = 64          # text tokens
II = 196         # image tokens
D = 512          # embed dim
NCORES = 8
IPC = B // NCORES          # images per core = 16
COLS = IPC * II            # 3136 sim columns per core
TEMP = 0.07
MT = (B * TT) // 128       # 64 text m-tiles
CT = 8                     # col tiles per core (392 cols each)
CW = 2 * II                # 392

MODE = "fp8dr"

_CACHE = {}


def _build(mode=MODE):
    import concourse.bacc as bacc
    import concourse.mybir as mybir
    from concourse import tile

    f32 = mybir.dt.float32
    X = mybir.AxisListType.X
    Exp = mybir.ActivationFunctionType.Exp
    Ln = mybir.ActivationFunctionType.Ln

    if mode == "fp8dr":
        mdt = mybir.dt.float8e4
        kch = 2           # two DoubleRow passes of K=256
        ksub = 2          # k-subtiles per pass
        perf = mybir.MatmulPerfMode.DoubleRow
    else:
        mdt = mybir.dt.bfloat16
        kch = 4
        ksub = 1
        perf = None

    nc = bacc.Bacc(
        "TRN2", target_bir_lowering=False, debug=False, num_devices=NCORES
    )

    txt_shape = [kch, 128, ksub, B * TT] if ksub > 1 else [kch, 128, B * TT]
    img_shape = [kch, 128, ksub, COLS] if ksub > 1 else [kch, 128, COLS]
    txt_dram = nc.dram_tensor("text_t", txt_shape, mdt, kind="ExternalInput")
    img_dram = nc.dram_tensor("img_t", img_shape, mdt, kind="ExternalInput")
    out_dram = nc.dram_tensor("loss", [1, 1], f32, kind="ExternalOutput")

    ident_np = np.eye(128, dtype=np.float32)
    ones2_np = np.zeros((128, 2), dtype=np.float32)
    ones2_np[0:64, 0] = 1.0 / (TT * TEMP)
    ones2_np[64:128, 1] = 1.0 / (TT * TEMP)
    half_np = np.full((128, 1), 0.5 / B, dtype=np.float32)
    ident_d = nc.inline_tensor(ident_np, "ident_c")
    ones2_d = nc.inline_tensor(ones2_np, "ones2_c")
    half_d = nc.inline_tensor(half_np, "half_c")

    with tile.TileContext(nc) as tc:
        with (
            tc.tile_pool(name="const", bufs=1) as constp,
            tc.tile_pool(name="data", bufs=1) as datap,
            tc.tile_pool(name="mx", bufs=1) as mxp,
            tc.tile_pool(name="work", bufs=1) as workp,
            tc.tile_pool(name="pmain", bufs=3, space="PSUM") as pmain,
            tc.tile_pool(name="pmisc", bufs=1, space="PSUM") as pmisc,
            tc.tile_pool(name="dram", bufs=1, space="DRAM") as dramp,
        ):
            ident = constp.tile([128, 128], f32, tag="ident", name="ident")
            nc.sync.dma_start(ident[:], ident_d[:])
            ones2 = constp.tile([128, 2], f32, tag="ones2", name="ones2")
            nc.sync.dma_start(ones2[:], ones2_d[:])
            half1 = constp.tile([128, 1], f32, tag="half1", name="half1")
            nc.sync.dma_start(half1[:], half_d[:])

            def data_tile(shape2, tagname):
                shape = [128] + ([ksub] if ksub > 1 else []) + [shape2]
                return datap.tile(shape, mdt, tag=tagname, name=tagname)

            def dma_in(t, dram, k, lo, hi):
                if ksub > 1:
                    nc.sync.dma_start(t[:], dram[k, :, :, lo:hi])
                else:
                    nc.sync.dma_start(t[:], dram[k, :, lo:hi])

            # resident tiles, DMA'd in first-use order: text g0, all image,
            # then remaining text (first m-tile needs txt g0 + all img h)
            txtt = {}
            imgt = {}
            for k in range(kch):
                t = data_tile(1024, f"txt{k}g0")
                dma_in(t, txt_dram, k, 0, 1024)
                txtt[k, 0] = t
            for h in range(CT):
                for k in range(kch):
                    t = data_tile(CW, f"img{k}h{h}")
                    dma_in(t, img_dram, k, CW * h, CW * (h + 1))
                    imgt[k, h] = t
            for g in range(1, 8):
                for k in range(kch):
                    t = data_tile(1024, f"txt{k}g{g}")
                    dma_in(t, txt_dram, k, 1024 * g, 1024 * (g + 1))
                    txtt[k, g] = t

            def lhsT_slice(k, g, mo):
                t = txtt[k, g]
                if ksub > 1:
                    return t[:, :, 128 * mo : 128 * (mo + 1)]
                return t[:, 128 * mo : 128 * (mo + 1)]

            maxv = mxp.tile([128, MT * IPC], f32, tag="maxv", name="maxv")
            # logits^T accumulator [16, 128]; written by interleaved mean-mms
            lgps = pmisc.tile([IPC, 128], f32, tag="misc", name="lgps")

            def mean_mm(m):
                # fold mean over t (and 1/T): [16,2] block of logits^T
                nc.tensor.matmul(
                    lgps[:, 2 * m : 2 * m + 2],
                    maxv[:, IPC * m : IPC * (m + 1)],
                    ones2[:],
                    start=True,
                    stop=True,
                )

            for m in range(MT):
                g, mo = divmod(m, 8)
                for cp in range(CT // 2):
                    # 2 full PSUM banks: 392-col image pair per bank (bank
                    # boundary at 512 f32 -- regions must not cross it)
                    ps = pmain.tile([128, 1024], f32, tag="ps", name="ps")
                    for r in range(2):
                        c = 2 * cp + r
                        for k in range(kch):
                            nc.tensor.matmul(
                                ps[:, 512 * r : 512 * r + CW],
                                lhsT_slice(k, g, mo),
                                imgt[k, c][:],
                                start=(k == 0),
                                stop=(k == kch - 1),
                                perf_mode=perf,
                            )
                    j = IPC * m + 4 * cp
                    nc.vector.reduce_max(
                        maxv[:, j : j + 4],
                        ps.rearrange("p (b r) -> p b r", b=2)[:, :, 0:CW]
                        .rearrange("p b (i x) -> p b i x", i=2),
                        axis=X,
                    )
                if m > 0:
                    mean_mm(m - 1)  # pipelined: maxv row-block m-1 is complete
            mean_mm(MT - 1)

            lgT_local = workp.tile([IPC, 128], f32, tag="lgT_local", name="lgT_local")
            nc.vector.tensor_copy(lgT_local[:], lgps[:])

            cc_in = dramp.tile([IPC, 128], f32, tag="cc_in", name="cc_in")
            cc_out = dramp.tile(
                [B, 128], f32, tag="cc_out", name="cc_out", addr_space="Shared"
            )
            nc.sync.dma_start(cc_in[:], lgT_local[:])
            nc.gpsimd.collective_compute(
                "AllGather",
                mybir.AluOpType.bypass,
                replica_groups=[list(range(NCORES))],
                ins=[cc_in.opt()],
                outs=[cc_out.opt()],
            )

            # full logits^T on every core -> scalar loss
            lgT = workp.tile([128, 128], f32, tag="lgT", name="lgT")
            nc.sync.dma_start(lgT[:], cc_out[:])
            ps_t = pmisc.tile([128, 128], f32, tag="misc", name="ps_t")
            nc.tensor.transpose(ps_t[:], lgT[:], ident[:])
            lg = workp.tile([128, 128], f32, tag="lg", name="lg")
            nc.vector.tensor_copy(lg[:], ps_t[:])

            def row_lse(src, nm):
                mx = workp.tile([128, 1], f32, tag=f"mx_{nm}", name=f"mx_{nm}")
                nc.vector.reduce_max(mx[:], src[:], axis=X)
                nmx = workp.tile([128, 1], f32, tag=f"nmx_{nm}", name=f"nmx_{nm}")
                nc.vector.tensor_scalar_mul(nmx[:], mx[:], -1.0)
                et = workp.tile([128, 128], f32, tag=f"et_{nm}", name=f"et_{nm}")
                sm = workp.tile([128, 1], f32, tag=f"sm_{nm}", name=f"sm_{nm}")
                nc.scalar.activation(
                    et[:], src[:], Exp, bias=nmx[:], scale=1.0, accum_out=sm[:]
                )
                ls = workp.tile([128, 1], f32, tag=f"ls_{nm}", name=f"ls_{nm}")
                nc.scalar.activation(ls[:], sm[:], Ln)
                lse = workp.tile([128, 1], f32, tag=f"lse_{nm}", name=f"lse_{nm}")
                nc.vector.tensor_sub(lse[:], ls[:], nmx[:])
                return lse

            lse_t2i = row_lse(lgT, "a")   # rows of logits^T: lse over b1
            lse_i2t = row_lse(lg, "b")    # rows of logits:   lse over b2

            dgt = workp.tile([128, 128], f32, tag="dgt", name="dgt")
            nc.vector.tensor_mul(dgt[:], lg[:], ident[:])
            dg = workp.tile([128, 1], f32, tag="dg", name="dg")
            nc.vector.reduce_sum(dg[:], dgt[:], axis=X)

            t_a = workp.tile([128, 1], f32, tag="t_a", name="t_a")
            nc.vector.tensor_add(t_a[:], lse_t2i[:], lse_i2t[:])
            t_b = workp.tile([128, 1], f32, tag="t_b", name="t_b")
            nc.vector.tensor_scalar_mul(t_b[:], dg[:], -2.0)
            rowterm = workp.tile([128, 1], f32, tag="rowterm", name="rowterm")
            nc.vector.tensor_add(rowterm[:], t_a[:], t_b[:])

            ps_l = pmisc.tile([1, 1], f32, tag="misc", name="ps_l")
            nc.tensor.matmul(ps_l[:], rowterm[:], half1[:], start=True, stop=True)
            loss_sb = workp.tile([1, 1], f32, tag="loss_sb", name="loss_sb")
            nc.vector.tensor_copy(loss_sb[:], ps_l[:])
            nc.sync.dma_start(out_dram[:], loss_sb[:])

    nc.compile()
    return nc


def _in_maps(image_tokens, text_tokens, mode=MODE):
    txt = np.asarray(text_tokens, dtype=np.float32).reshape(B * TT, D)
    txtT = np.ascontiguousarray(txt.T)  # [512, 8192]
    img = np.asarray(image_tokens, dtype=np.float32)

    if mode == "fp8dr":
        cast = ml_dtypes.float8_e4m3
        # d = kk*256 + j*128 + p  ->  [kk, p, j, cols] tile layout
        def prep(aT, n):
            a = aT.reshape(2, 2, 128, n).transpose(0, 2, 1, 3)
            return np.ascontiguousarray(a).astype(cast)
    else:
        cast = ml_dtypes.bfloat16

        def prep(aT, n):
            return np.ascontiguousarray(aT.reshape(4, 128, n)).astype(cast)

    text_t = prep(txtT, B * TT)
    maps = []
    for c in range(NCORES):
        sh = img[IPC * c : IPC * (c + 1)].reshape(COLS, D)
        shT = np.ascontiguousarray(sh.T)
        maps.append({"text_t": text_t, "img_t": prep(shT, COLS)})
    return maps


def run(image_tokens, text_tokens, trace=False):
    from concourse.bass_utils import run_bass_kernel_spmd

    if "nc" not in _CACHE:
        _CACHE["nc"] = _build()
    nc = _CACHE["nc"]
    res = run_bass_kernel_spmd(
        nc,
        _in_maps(image_tokens, text_tokens),
        core_ids=list(range(NCORES)),
        trace=trace,
    )
    return res


def kernel(image_tokens, text_tokens):
    res = run(image_tokens, text_tokens, trace=False)
    out = np.asarray(res.results[0]["loss"], dtype=np.float32).reshape(())
    return out
